# revision 1
# baseline (speedup 1.0000x reference)
"""Trainium2 Bass kernel for nn_CNN_Comp_29240137351522 (dense_cnn).

Math:  y = |IFFT_N( FFT_N(x)^2 * C )[255:2303]|,  C = FFT_N(w0)^2 * FFT_N(wl) / N
with N = 2560 >= 2559 so the chained full convolutions (x*w0, autoconv, *wl)
are exact linear convolutions.

Device decomposition (per core, data-parallel over batch):
  N = N2*N1, N1=128, N2=20;  time n = n2*128+n1,  freq k = k1*20+k2
  F1 (contract n2, PE, block-diag over n1 i-blocks of 4, twiddle folded)
  F3 (contract n1, PE, shared W128 DFT)          -> X[k1, (k2,b)]
  square (ACT/DVE fused into F3 eviction)         -> Zr = Xr^2-Xi^2, P = Xr*Xi
  I1 (contract k1, PE, per-k2 weights G = C-row-scaled inverse DFT; the
      factor 2 of Zi=2P folded into G variants), bf16
  I2 (contract k2, PE, block-diag over n1 i-blocks of {6,6,4}, twiddle folded,
      output n2 in [1,18)), bf16
  |.| fused into I2 eviction; raw tiles stored to DRAM, unscrambled on host.

Host does data movement only: batch shard, column permutation of x (so PE
transposes produce the (i,n2)-partition layout directly), and the inverse
row->output-column unscramble of the raw result.
"""

import numpy as np
import ml_dtypes

import concourse.bass as bass
import concourse.bacc as bacc
import concourse.mybir as mybir
from concourse.tile import TileContext
from concourse.bass_utils import run_bass_kernel_spmd

# ---------------- static problem config ----------------
B, NX = 4096, 1024
K0, KL = 129, 257
N = 2560
N1, N2 = 128, 20
NCORES = 8
BCORE = B // NCORES          # 512
CHUNK = 256
NCHUNKS = BCORE // CHUNK     # 2
N2OUT = 17                   # n2 in [1,18)
CROP0 = 255
CLASS_NUM = 2048
IBLK_I2 = (6, 6, 4)
JOFS_I2 = (0, 6, 12)
YRAW_ROWS = 8 * sum(IBLK_I2) * N2OUT  # 2176

f32 = mybir.dt.float32
f32r = mybir.dt.float32r
bf16 = mybir.dt.bfloat16
AO = mybir.AluOpType
AF = mybir.ActivationFunctionType


def _w(num, den):
    return np.exp(-2j * np.pi * np.asarray(num, np.float64) / den)


# ---------------- host-side constant arrays ----------------
def _build_consts():
    c = {}
    n1g = np.arange(N1)
    k1g = np.arange(N1)
    k2g = np.arange(N2)
    n2g8 = np.arange(8)

    # F1 lhsT: [128, 640]; block (g,jj) at partitions [32jj,32jj+32), cols [80g,80g+80)
    # rows (il in 4)*8 + n2, cols il*20 + k2; value W20[n2,k2] * W2560^{n1 k2}, n1=16g+4jj+il
    f1 = np.zeros((128, 640), np.complex128)
    for g in range(8):
        for jj in range(4):
            for il in range(4):
                n1 = 16 * g + 4 * jj + il
                blk = _w(np.outer(n2g8, k2g), N2) * _w(n1 * k2g, N)[None, :]
                f1[32 * jj + il * 8 : 32 * jj + il * 8 + 8, 80 * g + il * 20 : 80 * g + (il + 1) * 20] = blk
    c["cf1r"] = f1.real.astype(np.float32)
    c["cf1i"] = f1.imag.astype(np.float32)
    c["cf1n"] = (-f1.imag).astype(np.float32)

    # F3 lhsT (shared): W128[n1,k1]
    w3 = _w(np.outer(n1g, k1g), N1)
    c["cw3r"] = w3.real.astype(np.float32)
    c["cw3i"] = w3.imag.astype(np.float32)
    c["cw3n"] = (-w3.imag).astype(np.float32)

    # I1 base: W128i[k1,n1] (fp32, G built on device)
    wi = _w(-np.outer(k1g, n1g), N1)
    c["cwir"] = wi.real.astype(np.float32)
    c["cwii"] = wi.imag.astype(np.float32)

    # I2 lhsT: [120, 2176]; per (g,j) cols [off,off+M_j); block-diag il:
    # rows il*20+k2, cols il*17+(n2-1); value W20^{-k2 n2} * W2560^{-n1 k2}
    n2out = np.arange(1, 18)
    i2 = np.zeros((120, 2176), np.complex128)
    off = 0
    for g in range(8):
        for j, cnt in enumerate(IBLK_I2):
            for il in range(cnt):
                n1 = 16 * g + JOFS_I2[j] + il
                blk = _w(-np.outer(k2g, n2out), N2) * _w(-n1 * k2g, N)[:, None]
                i2[il * 20 : (il + 1) * 20, off + il * 17 : off + (il + 1) * 17] = blk
            off += cnt * N2OUT
    c["ci2r"] = i2.real.astype(ml_dtypes.bfloat16)
    c["ci2i"] = i2.imag.astype(ml_dtypes.bfloat16)
    c["ci2n"] = (-i2.imag).astype(ml_dtypes.bfloat16)

    # weight-DFT rhs constants
    nh = np.arange(128)
    t129 = _w(np.outer(nh, k2g), N)
    c["ct1r"] = t129.real.astype(np.float32)
    c["ct1i"] = t129.imag.astype(np.float32)
    t257b = _w(np.outer(nh, k2g), N) * _w(k2g, 20)[None, :]
    c["ct2r"] = t257b.real.astype(np.float32)
    c["ct2i"] = t257b.imag.astype(np.float32)
    t129e = _w(k2g, 20)
    c["te1r"] = t129e.real.astype(np.float32).reshape(1, N2)
    c["te1i"] = t129e.imag.astype(np.float32).reshape(1, N2)
    t257e = _w(k2g, 10)
    c["te2r"] = t257e.real.astype(np.float32).reshape(1, N2)
    c["te2i"] = t257e.imag.astype(np.float32).reshape(1, N2)

    c["ones1"] = np.ones((1, 128), np.float32)
    c["ident"] = np.eye(128, dtype=np.float32)
    return c


CONSTS = _build_consts()


def host_x_perm():
    """perm[g*128 + i*8 + n2] = n2*128 + 16g + i"""
    perm = np.empty(NX, np.int64)
    for g in range(8):
        for i in range(16):
            for n2 in range(8):
                perm[g * 128 + i * 8 + n2] = n2 * 128 + 16 * g + i
    return perm


def yraw_maps():
    """row r of yraw -> output column (n-255), valid mask."""
    rows = []
    for g in range(8):
        for j, cnt in enumerate(IBLK_I2):
            for il in range(cnt):
                n1 = 16 * g + JOFS_I2[j] + il
                for q in range(N2OUT):
                    rows.append((q + 1) * 128 + n1)
    narr = np.array(rows)
    valid = (narr >= CROP0) & (narr < CROP0 + CLASS_NUM)
    return narr, valid


XPERM = host_x_perm()
YN, YVALID = yraw_maps()


# ---------------- bass kernel builder ----------------
def build_nc():
    nc = bacc.Bacc("TRN2", target_bir_lowering=False, debug=False, num_devices=NCORES)

    # DRAM tensors
    d = {}
    d["xp_r"] = nc.dram_tensor("xp_r", [BCORE, NX], f32, kind="ExternalInput")
    d["xp_i"] = nc.dram_tensor("xp_i", [BCORE, NX], f32, kind="ExternalInput")
    for nm, shape in [("w0r", [K0]), ("w0i", [K0]), ("wlr", [KL]), ("wli", [KL])]:
        d[nm] = nc.dram_tensor(nm, shape, f32, kind="ExternalInput")
    cdt = {"cf1r": f32r, "cf1i": f32r, "cf1n": f32r,
           "cw3r": f32r, "cw3i": f32r, "cw3n": f32r,
           "ci2r": bf16, "ci2i": bf16, "ci2n": bf16,
           "ones1": f32r}
    for nm, arr in CONSTS.items():
        d[nm] = nc.dram_tensor(nm, list(arr.shape), cdt.get(nm, f32), kind="ExternalInput")
    yraw = nc.dram_tensor("yraw", [YRAW_ROWS, BCORE], f32, kind="ExternalOutput")

    with TileContext(nc) as tc:
        with (
            tc.tile_pool(name="cp", bufs=1) as cp,         # consts + persistent
            tc.tile_pool(name="bp", bufs=1) as bp,         # big per-chunk tiles
            tc.tile_pool(name="sp", bufs=6) as sp,         # small rotating tiles
            tc.tile_pool(name="tp", bufs=3) as tp,         # f32 tmp tiles
            tc.tile_pool(name="psa", bufs=2, space="PSUM") as psa,   # 4 tags x 2 bufs = 8 banks
        ):
            # ---- load constants ----
            ct = {}
            big_consts = {"ci2r", "ci2i", "ci2n", "cwir", "cwii"}
            for nm, arr in CONSTS.items():
                t = cp.tile(list(arr.shape), cdt.get(nm, f32), tag=nm)
                eng = nc.gpsimd if nm in big_consts else nc.sync
                eng.dma_start(out=t[:], in_=d[nm][:, :] if arr.ndim == 2 else d[nm][:])
                ct[nm] = t

            # ---- load w0/wl pieces as [128,1] / [1,1] columns ----
            wc = {}
            for nm, src, lo, hi in [
                ("w0r_c", "w0r", 0, 128), ("w0i_c", "w0i", 0, 128),
                ("wlr_c1", "wlr", 0, 128), ("wli_c1", "wli", 0, 128),
                ("wlr_c2", "wlr", 128, 256), ("wli_c2", "wli", 128, 256),
            ]:
                t = cp.tile([128, 1], f32, tag=nm)
                nc.sync.dma_start(out=t[:], in_=d[src][lo:hi])
                wc[nm] = t
            for nm, src, pos in [("w0r_e", "w0r", 128), ("w0i_e", "w0i", 128),
                                 ("wlr_e", "wlr", 256), ("wli_e", "wli", 256)]:
                t = cp.tile([1, 1], f32, tag=nm)
                nc.sync.dma_start(out=t[:], in_=d[src][pos:pos + 1])
                wc[nm] = t

            # ---- weight DFT: W0, WL [128, 20] ----
            def build_rhs(tr, ti, cr_, ci_, out_r, out_i):
                # out_r = tr*cr - ti*ci ; out_i = ti*cr + tr*ci   (complex (tr+i ti)*(cr+i ci))
                tmp = tp.tile([tr.shape[0], N2], f32, tag="wtmp")
                nc.vector.tensor_scalar(tmp[:], ti[:], ci_[:], None, AO.mult)
                nc.vector.scalar_tensor_tensor(out_r[:], tr[:], cr_[:], tmp[:], AO.mult, AO.subtract)
                tmp2 = tp.tile([tr.shape[0], N2], f32, tag="wtmp2")
                nc.vector.tensor_scalar(tmp2[:], tr[:], ci_[:], None, AO.mult)
                nc.vector.scalar_tensor_tensor(out_i[:], ti[:], cr_[:], tmp2[:], AO.mult, AO.add)

            def weight_dft(chunks, tail, out_r, out_i):
                """chunks: list of (t_r_tile, t_i_tile, colr, coli); tail: (te_r, te_i, er, ei)."""
                ps_r = psa.tile([128, N2], f32, tag="pAr")
                ps_i = psa.tile([128, N2], f32, tag="pAi")
                rhs = []
                for (t_r, t_i, colr, coli) in chunks:
                    rr = sp.tile([128, N2], f32r, tag="wrhs_r")
                    ri = sp.tile([128, N2], f32r, tag="wrhs_i")
                    build_rhs(t_r, t_i, colr, coli, rr, ri)
                    rhs.append((rr, ri))
                te_r, te_i, er, ei = tail
                tr = sp.tile([1, N2], f32r, tag="wtail_r")
                ti_ = sp.tile([1, N2], f32r, tag="wtail_i")
                tmp = tp.tile([1, N2], f32, tag="wtmp3")
                nc.vector.tensor_scalar(tmp[:], te_i[:], ei[:], None, AO.mult)
                nc.vector.scalar_tensor_tensor(tr[:], te_r[:], er[:], tmp[:], AO.mult, AO.subtract)
                tmp2 = tp.tile([1, N2], f32, tag="wtmp4")
                nc.vector.tensor_scalar(tmp2[:], te_r[:], ei[:], None, AO.mult)
                nc.vector.scalar_tensor_tensor(ti_[:], te_i[:], er[:], tmp2[:], AO.mult, AO.add)
                # psum groups
                first = True
                for (rr, ri) in rhs:
                    nc.tensor.matmul(ps_r[:], ct["cw3r"][:], rr[:], start=first, stop=False)
                    nc.tensor.matmul(ps_r[:], ct["cw3n"][:], ri[:], start=False, stop=False)
                    first = False
                nc.tensor.matmul(ps_r[:], ct["ones1"][:1, :], tr[:], start=False, stop=True)
                first = True
                for (rr, ri) in rhs:
                    nc.tensor.matmul(ps_i[:], ct["cw3i"][:], rr[:], start=first, stop=False)
                    nc.tensor.matmul(ps_i[:], ct["cw3r"][:], ri[:], start=False, stop=False)
                    first = False
                nc.tensor.matmul(ps_i[:], ct["ones1"][:1, :], ti_[:], start=False, stop=True)
                nc.vector.tensor_copy(out_r[:], ps_r[:])
                nc.vector.tensor_copy(out_i[:], ps_i[:])

            W0r = cp.tile([128, N2], f32, tag="W0r")
            W0i = cp.tile([128, N2], f32, tag="W0i")
            weight_dft(
                [(ct["ct1r"], ct["ct1i"], wc["w0r_c"], wc["w0i_c"])],
                (ct["te1r"], ct["te1i"], wc["w0r_e"], wc["w0i_e"]),
                W0r, W0i,
            )
            WLr = cp.tile([128, N2], f32, tag="WLr")
            WLi = cp.tile([128, N2], f32, tag="WLi")
            weight_dft(
                [(ct["ct1r"], ct["ct1i"], wc["wlr_c1"], wc["wli_c1"]),
                 (ct["ct2r"], ct["ct2i"], wc["wlr_c2"], wc["wli_c2"])],
                (ct["te2r"], ct["te2i"], wc["wlr_e"], wc["wli_e"]),
                WLr, WLi,
            )

            # ---- C = W0^2 * WL / N  [128, 20] ----
            Cr = cp.tile([128, N2], f32, tag="Cr")
            Ci = cp.tile([128, N2], f32, tag="Ci")
            ta = tp.tile([128, N2], f32, tag="ca")
            tb = tp.tile([128, N2], f32, tag="cb")
            tm1 = tp.tile([128, N2], f32, tag="cm1")
            tm2 = tp.tile([128, N2], f32, tag="cm2")
            nc.vector.tensor_mul(tm1[:], W0r[:], W0r[:])
            nc.vector.tensor_mul(tm2[:], W0i[:], W0i[:])
            nc.vector.tensor_sub(ta[:], tm1[:], tm2[:])          # a = W0r^2 - W0i^2
            nc.vector.tensor_mul(tm1[:], W0r[:], W0i[:])
            nc.vector.tensor_add(tb[:], tm1[:], tm1[:])          # b = 2 W0r W0i
            nc.vector.tensor_mul(tm1[:], ta[:], WLr[:])
            nc.vector.tensor_mul(tm2[:], tb[:], WLi[:])
            nc.vector.tensor_sub(tm1[:], tm1[:], tm2[:])
            nc.scalar.mul(Cr[:], tm1[:], 1.0 / N)
            nc.vector.tensor_mul(tm1[:], ta[:], WLi[:])
            nc.vector.tensor_mul(tm2[:], tb[:], WLr[:])
            nc.vector.tensor_add(tm1[:], tm1[:], tm2[:])
            nc.scalar.mul(Ci[:], tm1[:], 1.0 / N)

            # ---- G variants (bf16): G_k2 = C[:,k2] row-scaled W128i ----
            Gr = cp.tile([128, N2 * 128], bf16, tag="Gr")
            Gi = cp.tile([128, N2 * 128], bf16, tag="Gi")
            Gn2 = cp.tile([128, N2 * 128], bf16, tag="Gn2")  # -2*Gi
            Gr2 = cp.tile([128, N2 * 128], bf16, tag="Gr2")  # 2*Gr
            for k2 in range(N2):
                cr_ = Cr[:, k2 : k2 + 1]
                ci_ = Ci[:, k2 : k2 + 1]
                sl = slice(k2 * 128, (k2 + 1) * 128)
                gt = tp.tile([128, 128], f32, tag="gtmp")
                nc.vector.tensor_scalar(gt[:], ct["cwii"][:], ci_, None, AO.mult)
                nc.vector.scalar_tensor_tensor(Gr[:, sl], ct["cwir"][:], cr_, gt[:], AO.mult, AO.subtract)
                gt2 = tp.tile([128, 128], f32, tag="gtmp2")
                nc.vector.tensor_scalar(gt2[:], ct["cwir"][:], ci_, None, AO.mult)
                nc.vector.scalar_tensor_tensor(Gi[:, sl], ct["cwii"][:], cr_, gt2[:], AO.mult, AO.add)
                nc.scalar.mul(Gn2[:, sl], Gi[:, sl], -2.0)
                nc.scalar.mul(Gr2[:, sl], Gr[:, sl], 2.0)

            # ---- per-chunk pipeline ----
            i2_offs = []
            off = 0
            for g in range(8):
                for j, cnt in enumerate(IBLK_I2):
                    i2_offs.append((g, j, cnt, off))
                    off += cnt * N2OUT

            for c in range(NCHUNKS):
                # T-in: load + transpose
                xn_r = bp.tile([128, 2048], f32, tag="big1")
                xn_i = bp.tile([128, 2048], f32, tag="big2")
                for h in range(2):
                    rows = slice(c * CHUNK + h * 128, c * CHUNK + (h + 1) * 128)
                    nc.sync.dma_start(out=xn_r[:, h * 1024 : (h + 1) * 1024], in_=d["xp_r"][rows, :])
                    nc.sync.dma_start(out=xn_i[:, h * 1024 : (h + 1) * 1024], in_=d["xp_i"][rows, :])
                xt_r = bp.tile([128, 2048], f32r, tag="big3")
                xt_i = bp.tile([128, 2048], f32r, tag="big4")
                for plane, xn, xt in [(0, xn_r, xt_r), (1, xn_i, xt_i)]:
                    for h in range(2):
                        for g in range(8):
                            tps = psa.tile([128, 512], f32, tag="pBr")
                            nc.tensor.transpose(
                                tps[:128, :128],
                                xn[:, h * 1024 + g * 128 : h * 1024 + (g + 1) * 128],
                                ct["ident"][:],
                            )
                            nc.scalar.activation(
                                xt[:, g * 256 + h * 128 : g * 256 + (h + 1) * 128],
                                tps[:128, :128], AF.Copy,
                            )

                # F1 + pivot-C into plane-interleaved Abig [n1, k2*512 + plane*256 + b]
                Abig = bp.tile([128, 10240], f32r, tag="Abig")
                for g in range(8):
                    for jj in range(4):
                        pw = slice(32 * jj, 32 * jj + 32)
                        cwd = slice(80 * g, 80 * (g + 1))
                        rr = xt_r[pw, g * 256 : (g + 1) * 256]
                        ri = xt_i[pw, g * 256 : (g + 1) * 256]
                        lr = ct["cf1r"][pw, cwd]
                        li = ct["cf1i"][pw, cwd]
                        ln = ct["cf1n"][pw, cwd]
                        tpos = (32 * jj, 0)
                        pr = psa.tile([80, 256], f32, tag="pAr")
                        pi = psa.tile([80, 256], f32, tag="pAi")
                        nc.tensor.matmul(pr[:], lr, rr, start=True, stop=False, tile_position=tpos)
                        nc.tensor.matmul(pr[:], ln, ri, start=False, stop=True, tile_position=tpos)
                        nc.tensor.matmul(pi[:], li, rr, start=True, stop=False, tile_position=tpos)
                        nc.tensor.matmul(pi[:], lr, ri, start=False, stop=True, tile_position=tpos)
                        ag = sp.tile([80, 512], f32r, tag="ag")
                        nc.scalar.activation(ag[:, 0:256], pr[:], AF.Copy)
                        nc.vector.tensor_copy(ag[:, 256:512], pi[:])
                        # pivot: [(il,k2), (plane,b)] -> Abig[n1, k2*512+plane*256+b]
                        nc.sync.dma_start(
                            out=bass.AP(Abig.tensor, Abig[:].offset + (16 * g + 4 * jj) * 10240,
                                        [[10240, 4], [1, 10240]]),
                            in_=ag[:],
                        )

                # F3 + fused square eviction
                Zr = bp.tile([128, 5120], bf16, tag="Zr")
                Pt = bp.tile([128, 5120], bf16, tag="Pt")
                for k2 in range(N2):
                    asl_r = slice(k2 * 512, k2 * 512 + 256)
                    asl_i = slice(k2 * 512 + 256, k2 * 512 + 512)
                    zsl = slice(k2 * 256, (k2 + 1) * 256)
                    pr = psa.tile([128, 256], f32, tag="pBr")
                    pi = psa.tile([128, 256], f32, tag="pBi")
                    nc.tensor.matmul(pr[:], ct["cw3r"][:], Abig[:, asl_r], start=True, stop=False)
                    nc.tensor.matmul(pr[:], ct["cw3n"][:], Abig[:, asl_i], start=False, stop=True)
                    nc.tensor.matmul(pi[:], ct["cw3i"][:], Abig[:, asl_r], start=True, stop=False)
                    nc.tensor.matmul(pi[:], ct["cw3r"][:], Abig[:, asl_i], start=False, stop=True)
                    m1 = tp.tile([128, 256], f32, tag="sq1")
                    m2 = tp.tile([128, 256], f32, tag="sq2")
                    xi_s = tp.tile([128, 256], f32, tag="xis")
                    nc.vector.tensor_copy(xi_s[:], pi[:])
                    nc.scalar.activation(m1[:], pr[:], AF.Square)
                    nc.scalar.activation(m2[:], pi[:], AF.Square)
                    nc.vector.tensor_sub(Zr[:, zsl], m1[:], m2[:])
                    nc.vector.tensor_mul(Pt[:, zsl], pr[:], xi_s[:])

                # I1 (bf16); evict into plane-interleaved Ubig
                Ubig = bp.tile([128, 10240], bf16, tag="big1")
                for k2 in range(N2):
                    zsl = slice(k2 * 256, (k2 + 1) * 256)
                    gsl = slice(k2 * 128, (k2 + 1) * 128)
                    pr = psa.tile([128, 256], f32, tag="pAr")
                    pi = psa.tile([128, 256], f32, tag="pAi")
                    nc.tensor.matmul(pr[:], Gr[:, gsl], Zr[:, zsl], start=True, stop=False)
                    nc.tensor.matmul(pr[:], Gn2[:, gsl], Pt[:, zsl], start=False, stop=True)
                    nc.tensor.matmul(pi[:], Gi[:, gsl], Zr[:, zsl], start=True, stop=False)
                    nc.tensor.matmul(pi[:], Gr2[:, gsl], Pt[:, zsl], start=False, stop=True)
                    nc.scalar.activation(Ubig[:, k2 * 512 : k2 * 512 + 256], pr[:], AF.Copy)
                    nc.vector.tensor_copy(Ubig[:, k2 * 512 + 256 : (k2 + 1) * 512], pi[:])

                # pivot-D: one DMA per (g,j) into interleaved u2 [(il,k2), idx*512+plane*256+b]
                u2 = bp.tile([120, 24 * 512], bf16, tag="big2")
                for idx, (g, j, cnt, off) in enumerate(i2_offs):
                    n1_0 = 16 * g + JOFS_I2[j]
                    nc.sync.dma_start(
                        out=bass.AP(u2.tensor, u2[:].offset + idx * 512,
                                    [[24 * 512, cnt * 20], [1, 512]]),
                        in_=bass.AP(Ubig.tensor, Ubig[:].offset + n1_0 * 10240,
                                    [[10240, cnt], [1, 10240]]),
                    )

                # I2 (bf16) + fused abs + store
                for idx, (g, j, cnt, off) in enumerate(i2_offs):
                    Kj, Mj = cnt * 20, cnt * N2OUT
                    csl = slice(off, off + Mj)
                    usl_r = slice(idx * 512, idx * 512 + 256)
                    usl_i = slice(idx * 512 + 256, (idx + 1) * 512)
                    pr = psa.tile([102, 256], f32, tag="pBr")
                    pi = psa.tile([102, 256], f32, tag="pBi")
                    nc.tensor.matmul(pr[:Mj, :], ct["ci2r"][:Kj, csl], u2[:Kj, usl_r], start=True, stop=False)
                    nc.tensor.matmul(pr[:Mj, :], ct["ci2n"][:Kj, csl], u2[:Kj, usl_i], start=False, stop=True)
                    nc.tensor.matmul(pi[:Mj, :], ct["ci2i"][:Kj, csl], u2[:Kj, usl_r], start=True, stop=False)
                    nc.tensor.matmul(pi[:Mj, :], ct["ci2r"][:Kj, csl], u2[:Kj, usl_i], start=False, stop=True)
                    s1 = tp.tile([102, 256], f32, tag="ab1")
                    s2 = tp.tile([102, 256], f32, tag="ab2")
                    nc.scalar.activation(s1[:Mj, :], pr[:Mj, :], AF.Square)
                    nc.scalar.activation(s2[:Mj, :], pi[:Mj, :], AF.Square)
                    nc.vector.tensor_add(s1[:Mj, :], s1[:Mj, :], s2[:Mj, :])
                    ya = sp.tile([102, 256], f32, tag="yab")
                    nc.scalar.activation(ya[:Mj, :], s1[:Mj, :], AF.Sqrt)
                    nc.gpsimd.dma_start(
                        out=yraw[off : off + Mj, c * CHUNK : (c + 1) * CHUNK],
                        in_=ya[:Mj, :],
                    )

    nc.compile()
    return nc


_NC_CACHE = None
_LAST_IN_MAPS = None


def kernel(**inputs):
    global _NC_CACHE
    x_real = np.ascontiguousarray(inputs["x_real"], dtype=np.float32)
    x_imag = np.ascontiguousarray(inputs["x_imag"], dtype=np.float32)
    w0_real = np.ascontiguousarray(inputs["w0_real"], dtype=np.float32)
    w0_imag = np.ascontiguousarray(inputs["w0_imag"], dtype=np.float32)
    wl_real = np.ascontiguousarray(inputs["wl_real"], dtype=np.float32)
    wl_imag = np.ascontiguousarray(inputs["wl_imag"], dtype=np.float32)

    xp_r = x_real[:, XPERM]
    xp_i = x_imag[:, XPERM]

    const_maps = {}
    for nm, arr in CONSTS.items():
        const_maps[nm] = np.ascontiguousarray(arr)
    in_maps = []
    for cid in range(NCORES):
        rows = slice(cid * BCORE, (cid + 1) * BCORE)
        m = {
            "xp_r": np.ascontiguousarray(xp_r[rows]),
            "xp_i": np.ascontiguousarray(xp_i[rows]),
            "w0r": w0_real, "w0i": w0_imag,
            "wlr": wl_real, "wli": wl_imag,
        }
        m.update(const_maps)
        in_maps.append(m)

    global _LAST_IN_MAPS
    _LAST_IN_MAPS = in_maps
    if _NC_CACHE is None:
        _NC_CACHE = build_nc()
    res = run_bass_kernel_spmd(_NC_CACHE, in_maps, core_ids=list(range(NCORES)))

    out = np.empty((B, CLASS_NUM), np.float32)
    cols = YN[YVALID] - CROP0
    for cid in range(NCORES):
        yraw = res.results[cid]["yraw"]  # [2176, 512]
        out[cid * BCORE : (cid + 1) * BCORE, cols] = yraw[YVALID].T
    return out



# revision 37
# speedup vs baseline: 1.5932x; 1.5932x over previous
"""Trainium2 Bass kernel for nn_CNN_Comp_29240137351522 (dense_cnn), v2.

Math:  y = |IFFT_N( FFT_N(x)^2 * C )|,  C = FFT_N(w0)^2 * FFT_N(wl) / N
with N = 2304 (= 128*18).  2304 >= 2303 covers the autoconv h*h exactly, and
the final circular conv aliases y[n+2304] only onto n < 255, which the center
crop [255:2303) discards, so the cropped result is exact.

Device decomposition per core (data-parallel over batch, S = 512 samples):
  n = n2*128 + n1 (n2 in [0,18), x nonzero for n2 < 8),  k = 18*k1 + k2
  F1 (contract n2, block-diag over j = n1 mod 16, twiddle folded, bf16)
  pivot-C (DMA)   -> Abig[n1, (k2, plane, s)]
  F3 (contract n1, shared W128, bf16) -> X[k1, (k2, s)] in PSUM
  square (ACT dual-bank Square + DVE ops) -> Zr, P2 = 2*Xr*Xi (bf16)
  I1 (contract k1, G = C-row-scaled inverse DFT built on device, bf16)
  pivot-D (DMA)   -> u2[(j, k2), (plane, s)]
  I2 (contract k2, block-diag over j, bf16) + |.|^2 + sqrt -> yraw (bf16)
Host does data movement only: batch shard, x permutation into the F1-ready
layout, packing of weight vectors, and the output unscramble.
"""

import numpy as np
import ml_dtypes

import concourse.bass as bass
import concourse.bacc as bacc
import concourse.mybir as mybir
from concourse.tile import TileContext
from concourse.bass_utils import run_bass_kernel_spmd

# ---------------- static problem config ----------------
B, NX = 4096, 1024
K0, KL = 129, 257
N = 2304
N1, N2 = 128, 18
NCORES = 8
S = B // NCORES              # 512 samples per core, single chunk
CROP0 = 255
CLASS_NUM = 2048
K2SPLIT = ((0, 8), (8, 16), (16, 18))     # F1 column splits (k2-major)
JBLK = ((0, 7), (7, 14), (14, 16))        # I2 j-blocks per g
F1COLS = 288                               # 18*16 cols per g
I2COLS = 272                               # 16*17 cols per g
YROWS = 119                                # max I2 out rows (7*17)

f32 = mybir.dt.float32
f32r = mybir.dt.float32r
bf16 = mybir.dt.bfloat16
AO = mybir.AluOpType
AF = mybir.ActivationFunctionType

BF = ml_dtypes.bfloat16


def _w(num, den):
    return np.exp(-2j * np.pi * np.asarray(num, np.float64) / den)


# ---------------- host-side constant arrays ----------------
def _build_consts():
    c = {}
    n1g = np.arange(N1)
    k1g = np.arange(N1)
    k2g = np.arange(N2)

    # F1 lhsT [128, 8*288]: row p = 8j + n2 ; col g*288 + sbase + k2sub*16 + j
    # value W18^{n2 k2} * W2304^{(16g+j) k2}
    f1 = np.zeros((128, 8 * F1COLS), np.complex128)
    for g in range(8):
        for (k2lo, k2hi), sbase in zip(K2SPLIT, (0, 128, 256)):
            nk = k2hi - k2lo
            for k2 in range(k2lo, k2hi):
                for j in range(16):
                    n1 = 16 * g + j
                    col = g * F1COLS + sbase + j * nk + (k2 - k2lo)
                    vals = _w(np.arange(8) * k2, N2) * _w(n1 * k2, N)
                    f1[8 * j : 8 * j + 8, col] = vals
    c["cf1"] = np.concatenate(
        [f1.real, f1.imag, -f1.imag], axis=1).astype(BF)   # [128, 3*2304]

    # F3 lhsT (shared): W128[n1,k1], bf16 + f32 copy for the weight-DFT mms
    w3 = _w(np.outer(n1g, k1g), N1)
    w3cat = np.concatenate([w3.real, w3.imag, -w3.imag], axis=1)
    c["cw3"] = w3cat.astype(BF)                            # [128, 384]
    c["cw3f"] = w3cat.astype(np.float32)                   # [128, 384]

    # inverse-DFT base tiled over k2, divided by N (folds the 1/N of C):
    # cwiB[:, v*2304 + k2*128 + p] = {Re,Im}(W128^{-k1 p}) / N
    wi = _w(-np.outer(k1g, n1g), N1) / N
    blk = np.concatenate([np.tile(wi.real, (1, N2)), np.tile(wi.imag, (1, N2))], axis=1)
    c["cwiB"] = blk.astype(BF)                             # [128, 2*2304]

    # I2 lhsT [128, 8*272]: per (g, jb): rows p = j'*18 + k2, col g*272 + base
    # + j'*17 + (q-1); value W18^{-q k2} * W2304^{-(16g+j0+j') k2}, q in [1,18)
    i2 = np.zeros((128, 8 * I2COLS), np.complex128)
    qg = np.arange(1, 18)
    for g in range(8):
        base = 0
        for (j0, j1) in JBLK:
            for jp in range(j1 - j0):
                n1 = 16 * g + j0 + jp
                blkv = _w(-np.outer(k2g, qg), N2) * _w(-n1 * k2g, N)[:, None]
                rows = slice(jp * 18, jp * 18 + 18)
                cols = slice(g * I2COLS + base + jp * 17, g * I2COLS + base + (jp + 1) * 17)
                i2[rows, cols] = blkv
            base += (j1 - j0) * 17
    c["ci2"] = np.concatenate(
        [i2.real, i2.imag, -i2.imag], axis=1).astype(BF)   # [128, 3*2176]

    # weight-DFT rhs constants (f32), packed into one [128, 272] tensor:
    # cols 0:18 ct1r | 18:36 ct1i | 36:54 ct2r | 54:72 ct2i
    # row0 cols 72:90 te1r | 90:108 te1i | 108:126 te2r | 126:144 te2i
    # row0 cols 144:272 ones (128)
    nh = np.arange(128)
    sm = np.zeros((128, 272), np.float32)
    t1 = _w(np.outer(nh, k2g), N)
    sm[:, 0:18] = t1.real
    sm[:, 18:36] = t1.imag
    t2 = _w(np.outer(nh, k2g), N) * _w(k2g, N2)[None, :]
    sm[:, 36:54] = t2.real
    sm[:, 54:72] = t2.imag
    te1 = _w(k2g, N2)
    sm[0, 72:90] = te1.real
    sm[0, 90:108] = te1.imag
    te2 = _w(k2g, 9)
    sm[0, 108:126] = te2.real
    sm[0, 126:144] = te2.imag
    sm[0, 144:272] = 1.0
    c["csm"] = sm

    return c


CONSTS = _build_consts()


# ---------------- bass kernel builder ----------------
def build_nc():
    nc = bacc.Bacc("TRN2", target_bir_lowering=False, debug=False, num_devices=NCORES)

    d = {}
    d["xt"] = nc.dram_tensor("xt", [128, 2 * 4096], bf16, kind="ExternalInput")
    d["wpack"] = nc.dram_tensor("wpack", [128, 10], f32, kind="ExternalInput")
    cdt = {"cw3f": f32, "csm": f32}
    for nm, arr in CONSTS.items():
        d[nm] = nc.dram_tensor(nm, list(arr.shape), cdt.get(nm, bf16), kind="ExternalInput")
    yraw = nc.dram_tensor("yraw", [YROWS, 8 * 1536], bf16, kind="ExternalOutput")

    with TileContext(nc) as tc:
        with (
            tc.tile_pool(name="cp", bufs=1) as cp,          # persistent consts
            tc.tile_pool(name="bp", bufs=1) as bp,          # Abig / Ubig / G
            tc.tile_pool(name="sp", bufs=3) as sp,          # rotating stage tiles
            tc.tile_pool(name="gp", bufs=2) as gp,          # G-build temporaries
            tc.tile_pool(name="stp", bufs=4) as stp,        # pivot-C staging
            tc.tile_pool(name="xp2", bufs=2) as xp2,        # xi/y copies
            tc.tile_pool(name="up", bufs=3) as up,          # u2 tiles
            tc.tile_pool(name="yp", bufs=4) as yp,          # yy tiles
            tc.tile_pool(name="zp", bufs=4) as zp,          # z tiles
            tc.tile_pool(name="tp", bufs=1) as tp,          # small f32 tmps
            tc.tile_pool(name="psa", bufs=2, space="PSUM") as psa,
        ):
            # ---- const + input DMAs (sync engine; ordered by need) ----
            # xt free layout: g*1024 + plane*512 + s  (per-g slices contiguous)
            wpk = cp.tile([128, 10], f32, tag="wpack")
            nc.sync.dma_start(out=wpk[:], in_=d["wpack"][:, :])
            csm = cp.tile([128, 272], f32, tag="csm")
            nc.sync.dma_start(out=csm[:], in_=d["csm"][:, :])
            cw3f = cp.tile([128, 384], f32, tag="cw3f")
            nc.sync.dma_start(out=cw3f[:], in_=d["cw3f"][:, :])
            cf1 = cp.tile([128, 3 * 2304], bf16, tag="cf1")
            for v in (0, 2, 1):
                vs = slice(v * 2304, (v + 1) * 2304)
                nc.sync.dma_start(out=cf1[:, vs], in_=d["cf1"][:, vs])
            xt = cp.tile([128, 8192], bf16, tag="xt")
            for g in range(8):
                gs = slice(g * 1024, (g + 1) * 1024)
                nc.sync.dma_start(out=xt[:, gs], in_=d["xt"][:, gs])
            cw3 = cp.tile([128, 384], bf16, tag="cw3")
            nc.sync.dma_start(out=cw3[:], in_=d["cw3"][:, :])
            cwiB = cp.tile([128, 2 * 2304], bf16, tag="cwiB")
            nc.sync.dma_start(out=cwiB[:], in_=d["cwiB"][:, :])
            ci2 = cp.tile([128, 3 * 2176], bf16, tag="ci2")

            # ---- weight DFT -> C (without 1/N; folded into cwiB) ----
            # rhs builds [rows, 18] f32  (complex products via DVE small ops)
            def cplx_rhs(rows, tr, ti, cr, ci, outr, outi):
                # (cr + i ci) * (tr + i ti); cr/ci are [rows,1] scalar APs
                t = tp.tile([128, 18], f32, tag="wtmp")
                nc.vector.tensor_scalar(t[:rows, :], ti, ci, None, AO.mult)
                nc.vector.scalar_tensor_tensor(outr, tr, cr, t[:rows, :], AO.mult, AO.subtract)
                t2 = tp.tile([128, 18], f32, tag="wtmp2")
                nc.vector.tensor_scalar(t2[:rows, :], tr, ci, None, AO.mult)
                nc.vector.scalar_tensor_tensor(outi, ti, cr, t2[:rows, :], AO.mult, AO.add)

            rhs0 = tp.tile([128, 36], f32, tag="rhs0")
            cplx_rhs(128, csm[:, 0:18], csm[:, 18:36], wpk[:, 0:1], wpk[:, 1:2],
                     rhs0[:, 0:18], rhs0[:, 18:36])
            tl0 = tp.tile([1, 36], f32, tag="tl0")
            cplx_rhs(1, csm[0:1, 72:90], csm[0:1, 90:108], wpk[0:1, 6:7], wpk[0:1, 7:8],
                     tl0[:, 0:18], tl0[:, 18:36])
            rhs1 = tp.tile([128, 36], f32, tag="rhs1")
            cplx_rhs(128, csm[:, 0:18], csm[:, 18:36], wpk[:, 2:3], wpk[:, 3:4],
                     rhs1[:, 0:18], rhs1[:, 18:36])
            rhs2 = tp.tile([128, 36], f32, tag="rhs2")
            cplx_rhs(128, csm[:, 36:54], csm[:, 54:72], wpk[:, 4:5], wpk[:, 5:6],
                     rhs2[:, 0:18], rhs2[:, 18:36])
            tl2 = tp.tile([1, 36], f32, tag="tl2")
            cplx_rhs(1, csm[0:1, 108:126], csm[0:1, 126:144], wpk[0:1, 8:9], wpk[0:1, 9:10],
                     tl2[:, 0:18], tl2[:, 18:36])

            w3fr = cw3f[:, 0:128]
            w3fi = cw3f[:, 128:256]
            w3fn = cw3f[:, 256:384]
            onesf = csm[0:1, 144:272]

            w0ps = psa.tile([128, 36], f32, tag="pX")
            nc.tensor.matmul(w0ps[:, 0:18], w3fr, rhs0[:, 0:18], start=True, stop=False)
            nc.tensor.matmul(w0ps[:, 0:18], w3fn, rhs0[:, 18:36], start=False, stop=False)
            nc.tensor.matmul(w0ps[:, 0:18], onesf, tl0[:, 0:18], start=False, stop=True)
            nc.tensor.matmul(w0ps[:, 18:36], w3fi, rhs0[:, 0:18], start=True, stop=False)
            nc.tensor.matmul(w0ps[:, 18:36], w3fr, rhs0[:, 18:36], start=False, stop=False)
            nc.tensor.matmul(w0ps[:, 18:36], onesf, tl0[:, 18:36], start=False, stop=True)
            wlps = psa.tile([128, 36], f32, tag="pU")
            nc.tensor.matmul(wlps[:, 0:18], w3fr, rhs1[:, 0:18], start=True, stop=False)
            nc.tensor.matmul(wlps[:, 0:18], w3fn, rhs1[:, 18:36], start=False, stop=False)
            nc.tensor.matmul(wlps[:, 0:18], w3fr, rhs2[:, 0:18], start=False, stop=False)
            nc.tensor.matmul(wlps[:, 0:18], w3fn, rhs2[:, 18:36], start=False, stop=False)
            nc.tensor.matmul(wlps[:, 0:18], onesf, tl2[:, 0:18], start=False, stop=True)
            nc.tensor.matmul(wlps[:, 18:36], w3fi, rhs1[:, 0:18], start=True, stop=False)
            nc.tensor.matmul(wlps[:, 18:36], w3fr, rhs1[:, 18:36], start=False, stop=False)
            nc.tensor.matmul(wlps[:, 18:36], w3fi, rhs2[:, 0:18], start=False, stop=False)
            nc.tensor.matmul(wlps[:, 18:36], w3fr, rhs2[:, 18:36], start=False, stop=False)
            nc.tensor.matmul(wlps[:, 18:36], onesf, tl2[:, 18:36], start=False, stop=True)

            w0sb = tp.tile([128, 36], f32, tag="w0sb")
            nc.scalar.activation(w0sb[:], w0ps[:], AF.Copy)
            wlsb = tp.tile([128, 36], f32, tag="wlsb")
            nc.scalar.activation(wlsb[:], wlps[:], AF.Copy)

            # C*N = W0^2 * WL  (f32, [128, 18] each)
            ca = tp.tile([128, 18], f32, tag="ca")
            cb = tp.tile([128, 18], f32, tag="cb")
            cm1 = tp.tile([128, 18], f32, tag="cm1")
            cm2 = tp.tile([128, 18], f32, tag="cm2")
            nc.vector.tensor_mul(cm1[:], w0sb[:, 0:18], w0sb[:, 0:18])
            nc.vector.tensor_mul(cm2[:], w0sb[:, 18:36], w0sb[:, 18:36])
            nc.vector.tensor_sub(ca[:], cm1[:], cm2[:])
            nc.vector.scalar_tensor_tensor(cb[:], w0sb[:, 0:18], 2.0, w0sb[:, 18:36],
                                           AO.mult, AO.mult)
            crn = tp.tile([128, 18], f32, tag="crn")
            cin = tp.tile([128, 18], f32, tag="cin")
            nc.vector.tensor_mul(cm1[:], ca[:], wlsb[:, 0:18])
            nc.vector.tensor_mul(cm2[:], cb[:], wlsb[:, 18:36])
            nc.vector.tensor_sub(crn[:], cm1[:], cm2[:])
            nc.vector.tensor_mul(cm1[:], ca[:], wlsb[:, 18:36])
            nc.vector.tensor_mul(cm2[:], cb[:], wlsb[:, 0:18])
            nc.vector.tensor_add(cin[:], cm1[:], cm2[:])

            # ---- G build: G = (wi/N) * C*N, [128, 2304] bf16 x3 ----
            # per-k2 pieces on DVE or Pool (SBUF-only, hw-legal); emitted
            # interleaved into phase A so they don't block phase-A evictions
            Gr = bp.tile([128, 2304], bf16, tag="Gr")
            Gi = bp.tile([128, 2304], bf16, tag="Gi")
            Gn = bp.tile([128, 2304], bf16, tag="Gn")
            wbr = cwiB[:, 0:2304]
            wbi = cwiB[:, 2304:4608]

            def g_piece(k2, eng):
                v = nc.vector if eng == "dve" else nc.gpsimd
                ksl = slice(k2 * 128, (k2 + 1) * 128)
                crc = crn[:, k2:k2 + 1]
                cic = cin[:, k2:k2 + 1]
                gA = gp.tile([128, 128], bf16, tag="gA")
                gB = gp.tile([128, 128], bf16, tag="gB")
                v.tensor_scalar(gA[:], wbr[:, ksl], crc, None, AO.mult)
                v.tensor_scalar(gB[:], wbi[:, ksl], cic, None, AO.mult)
                v.tensor_tensor(Gr[:, ksl], gA[:], gB[:], AO.subtract)
                gA2 = gp.tile([128, 128], bf16, tag="gA")
                gB2 = gp.tile([128, 128], bf16, tag="gB")
                v.tensor_scalar(gA2[:], wbi[:, ksl], crc, None, AO.mult)
                v.tensor_scalar(gB2[:], wbr[:, ksl], cic, None, AO.mult)
                v.tensor_tensor(Gi[:, ksl], gA2[:], gB2[:], AO.add)
                v.tensor_scalar(Gn[:, ksl], Gi[:, ksl], -1.0, None, AO.mult)

            cf1r = cf1[:, 0:2304]
            cf1i = cf1[:, 2304:4608]
            cf1n = cf1[:, 4608:6912]
            w3r = cw3[:, 0:128]
            w3i = cw3[:, 128:256]
            w3n = cw3[:, 256:384]

            Abig = bp.tile([128, 18432], bf16, tag="Abig")
            Ubig = bp.tile([128, 18432], bf16, tag="Ubig")

            def ev_op(engine, dst, src):
                # pool/gpsimd cannot access PSUM on TRN2 hardware
                if engine == "act":
                    nc.scalar.activation(dst, src, AF.Copy)
                else:
                    nc.vector.tensor_copy(dst, src)

            # ---- Phase A: F1 + pivot-C (evict/DMA lagged one iteration) ----
            # DVE runs the G-build and Pool/ACT the crE expansion early; keep
            # phase-A evictions off DVE entirely (in-order queues would stall
            # F1 behind the G-build) and give Pool only the tail.
            EVA = ["act", "act", "act", "dve", "act", "dve", "act", "dve",
                   "act", "dve", "act", "dve", "act", "dve", "act", "dve",
                   "act", "dve", "act", "dve", "act", "dve", "act", "act"]
            # G pieces: (emit-after-iteration, k2, engine)
            GSCHED = {}
            _gk2 = 0
            for _it in range(2, 24):
                if _it % 3 != 1:
                    GSCHED[_it] = (_gk2, "pool" if _it % 3 == 2 else "dve")
                    _gk2 += 1
            # remaining k2 values flushed after phase A
            GREST = list(range(_gk2, N2))
            fa_pend = []  # (psum, rows, g, k2lo, nk)

            def fa_flush(idx):
                ab, rows, g, k2lo, nk = fa_pend[idx]
                stg = stp.tile([128, 1024], bf16, tag="stg")
                ev_op(EVA[idx], stg[:rows, :], ab[:rows, :])
                nc.sync.dma_start(
                    out=bass.AP(Abig.tensor,
                                Abig[:].offset + (16 * g) * 18432 + k2lo * 1024,
                                [[18432, 16], [1024, nk], [1, 1024]]),
                    in_=bass.AP(stg.tensor, stg[:].offset,
                                [[1024, 16 * nk], [1, 1024]]),
                )

            it = 0
            for si, (k2lo, k2hi) in enumerate(K2SPLIT):
                nk = k2hi - k2lo
                rows = nk * 16
                sbase = si * 128
                for g in range(8):
                    csl = slice(g * F1COLS + sbase, g * F1COLS + sbase + rows)
                    xr = xt[:, g * 1024 : g * 1024 + 512]
                    xi = xt[:, g * 1024 + 512 : (g + 1) * 1024]
                    ab = psa.tile([128, 1024], f32, tag="pX" if it % 2 == 0 else "pU")
                    nc.tensor.matmul(ab[:rows, 0:512], cf1r[:, csl], xr, start=True, stop=False)
                    nc.tensor.matmul(ab[:rows, 0:512], cf1n[:, csl], xi, start=False, stop=True)
                    nc.tensor.matmul(ab[:rows, 512:1024], cf1i[:, csl], xr, start=True, stop=False)
                    nc.tensor.matmul(ab[:rows, 512:1024], cf1r[:, csl], xi, start=False, stop=True)
                    fa_pend.append((ab, rows, g, k2lo, nk))
                    if it >= 1:
                        fa_flush(it - 1)
                    if it in GSCHED:
                        g_piece(*GSCHED[it])
                    it += 1
            fa_flush(it - 1)
            for _k2 in GREST:
                g_piece(_k2, "dve" if _k2 % 2 == 0 else "pool")
            # late const loads: needed only in phase C
            for v in range(3):
                vs = slice(v * 2176, (v + 1) * 2176)
                nc.sync.dma_start(out=ci2[:, vs], in_=d["ci2"][:, vs])

            # ---- Phase B: F3 + square + I1, software-pipelined by k2 ----
            # per-iteration emit: F3(k2), I1(k2-1), sq(k2), P2(k2), Zr(k2), ev(k2-1)
            XCE = ["act", "dve", "dve", "act", "dve", "act", "dve", "dve", "act",
                   "dve", "act", "dve", "dve", "act", "dve", "act", "dve", "act"]
            ZRE = ["pool", "pool", "dve", "pool", "pool", "dve", "pool", "pool",
                   "dve", "pool", "pool", "dve", "pool", "pool", "dve", "pool",
                   "pool", "dve"]
            EVB = ["act", "dve", "dve", "act", "act", "dve", "dve", "act", "dve",
                   "act", "dve", "dve", "act", "act", "dve", "dve", "act", "act"]
            zt = [None] * N2
            upst = [None] * N2

            def b_f3(k2):
                ar = Abig[:, k2 * 1024 : k2 * 1024 + 512]
                ai = Abig[:, k2 * 1024 + 512 : (k2 + 1) * 1024]
                xps = psa.tile([128, 1024], f32, tag="pX")
                nc.tensor.matmul(xps[:, 0:512], w3r, ar, start=True, stop=False)
                nc.tensor.matmul(xps[:, 0:512], w3n, ai, start=False, stop=True)
                nc.tensor.matmul(xps[:, 512:1024], w3i, ar, start=True, stop=False)
                nc.tensor.matmul(xps[:, 512:1024], w3r, ai, start=False, stop=True)
                return xps

            def b_i1(k2):
                z = zt[k2]
                gsl = slice(k2 * 128, (k2 + 1) * 128)
                ups = psa.tile([128, 1024], f32, tag="pU")
                nc.tensor.matmul(ups[:, 0:512], Gr[:, gsl], z[:, 0:512], start=True, stop=False)
                nc.tensor.matmul(ups[:, 0:512], Gn[:, gsl], z[:, 512:1024], start=False, stop=True)
                nc.tensor.matmul(ups[:, 512:1024], Gi[:, gsl], z[:, 0:512], start=True, stop=False)
                nc.tensor.matmul(ups[:, 512:1024], Gr[:, gsl], z[:, 512:1024], start=False, stop=True)
                upst[k2] = ups

            sqt = [None] * N2

            def b_sqp2(k2, xps):
                # hw: only one non-scalar input may be in PSUM; squares go
                # through ACT (single input), the cross product via an SBUF
                # copy of Xi
                sq = sp.tile([128, 1024], bf16, tag="sq")
                nc.scalar.activation(sq[:], xps[:], AF.Square)
                xic = xp2.tile([128, 512], bf16, tag="xic")
                if XCE[k2] == "act":
                    nc.scalar.activation(xic[:], xps[:, 512:1024], AF.Copy)
                else:
                    nc.vector.tensor_copy(xic[:], xps[:, 512:1024])
                sqt[k2] = sq
                z = zp.tile([128, 1024], bf16, tag="z")
                nc.vector.scalar_tensor_tensor(z[:, 512:1024], xps[:, 0:512], 2.0,
                                               xic[:], AO.mult, AO.mult)
                zt[k2] = z

            def b_zr(k2):
                v = nc.gpsimd if ZRE[k2] == "pool" else nc.vector
                v.tensor_tensor(zt[k2][:, 0:512], sqt[k2][:, 0:512], sqt[k2][:, 512:1024],
                                AO.subtract)

            def b_ev(k2):
                ev_op(EVB[k2], Ubig[:, k2 * 1024 : (k2 + 1) * 1024], upst[k2][:])

            for k2 in range(N2):
                xps = b_f3(k2)
                if k2 >= 2:
                    b_i1(k2 - 2)
                b_sqp2(k2, xps)
                if k2 >= 1:
                    b_zr(k2 - 1)
                if k2 >= 3:
                    b_ev(k2 - 3)
            b_zr(N2 - 1)
            for k2 in (N2 - 2, N2 - 1):
                b_i1(k2)
            for k2 in (N2 - 3, N2 - 2, N2 - 1):
                b_ev(k2)

            # ---- Phase C: pivot-D + I2 + |.| + store, pipelined by (g, blk) ----
            ci2r = ci2[:, 0:2176]
            ci2i = ci2[:, 2176:4352]
            ci2n = ci2[:, 4352:6528]
            POSTE = ["act", "act", "dve", "act", "dve", "dve"] * 4
            ADDE = ["pool", "pool", "dve"] * 8

            def c_pivd(g):
                tiles = []
                for bi, (j0, j1) in enumerate(JBLK):
                    cnt = j1 - j0
                    u2 = up.tile([128, 1024], bf16, tag=f"u2{bi}")
                    nc.sync.dma_start(
                        out=bass.AP(u2.tensor, u2[:].offset,
                                    [[1024, cnt * 18], [1, 1024]]),
                        in_=bass.AP(Ubig.tensor,
                                    Ubig[:].offset + (16 * g + j0) * 18432,
                                    [[18432, cnt], [1024, 18], [1, 1024]]),
                    )
                    tiles.append(u2)
                return tiles

            CB = [(0, 0, 119), (1, 119, 119), (2, 238, 34)]  # (blk, colbase, rows)

            def c_mm(g, bi, u2):
                j0, j1 = JBLK[bi]
                cnt = j1 - j0
                rows = CB[bi][2]
                parts = cnt * 18
                csl = slice(g * I2COLS + CB[bi][1], g * I2COLS + CB[bi][1] + rows)
                yps = psa.tile([128, 1024], f32, tag="pX" if (g + bi) % 2 == 0 else "pU")
                nc.tensor.matmul(yps[:rows, 0:512], ci2r[:parts, csl], u2[:parts, 0:512],
                                 start=True, stop=False)
                nc.tensor.matmul(yps[:rows, 0:512], ci2n[:parts, csl], u2[:parts, 512:1024],
                                 start=False, stop=True)
                nc.tensor.matmul(yps[:rows, 512:1024], ci2i[:parts, csl], u2[:parts, 0:512],
                                 start=True, stop=False)
                nc.tensor.matmul(yps[:rows, 512:1024], ci2r[:parts, csl], u2[:parts, 512:1024],
                                 start=False, stop=True)
                return yps

            def c_post(g, bi, yps, yy):
                rows = CB[bi][2]
                m = sp.tile([128, 1024], bf16, tag="m")
                pe = POSTE[(g * 3 + bi) % len(POSTE)]
                if pe == "act":
                    nc.scalar.activation(m[:rows, :], yps[:rows, :], AF.Square)
                else:
                    mc = xp2.tile([128, 1024], bf16, tag="mc")
                    nc.vector.tensor_copy(mc[:rows, :], yps[:rows, :])
                    nc.vector.tensor_mul(m[:rows, :], mc[:rows, :], mc[:rows, :])
                va = nc.gpsimd if ADDE[(g * 3 + bi) % len(ADDE)] == "pool" else nc.vector
                va.tensor_tensor(yy[:rows, bi * 512 : (bi + 1) * 512],
                                 m[:rows, 0:512], m[:rows, 512:1024], AO.add)

            # steady state per g: pivD(g+2), mm(g,*) with post lag 1 blk,
            # sqrt(g-1) right after post(g-1,2), out-DMA(g-2) last (so the
            # SP queue never parks on an unmet sqrt wait ahead of pivDs).
            u2_0 = c_pivd(0)
            u2_1 = c_pivd(1)
            u2_t = {0: u2_0, 1: u2_1}
            yy_t = {}
            pend = []  # (g, bi, yps)
            fin = []   # g values whose posts are all emitted
            for g in range(8):
                yyg = yp.tile([128, 1536], bf16, tag="yy")
                nc.gpsimd.memset(yyg[0:YROWS, 1024:1536], 0.0)
                yy_t[g] = yyg
                if g + 2 < 8:
                    u2_t[g + 2] = c_pivd(g + 2)
                for bi in range(3):
                    yps = c_mm(g, bi, u2_t[g][bi])
                    if len(pend) >= 2:
                        pg, pbi, pyps = pend.pop(0)
                        c_post(pg, pbi, pyps, yy_t[pg])
                        if pbi == 2:
                            nc.scalar.activation(yy_t[pg][:YROWS, :], yy_t[pg][:YROWS, :], AF.Sqrt)
                            fin.append(pg)
                    pend.append((g, bi, yps))
                if len(fin) >= 2:
                    og = fin.pop(0)
                    nc.sync.dma_start(
                        out=yraw[0:YROWS, og * 1536 : (og + 1) * 1536],
                        in_=yy_t[og][:YROWS, :],
                    )
            while pend:
                pg, pbi, pyps = pend.pop(0)
                c_post(pg, pbi, pyps, yy_t[pg])
                if pbi == 2:
                    nc.scalar.activation(yy_t[pg][:YROWS, :], yy_t[pg][:YROWS, :], AF.Sqrt)
                    fin.append(pg)
            for og in fin:
                nc.sync.dma_start(
                    out=yraw[0:YROWS, og * 1536 : (og + 1) * 1536],
                    in_=yy_t[og][:YROWS, :],
                )

    nc.compile()
    return nc


_NC_CACHE = None


# ---------------- host-side orchestration ----------------
def _host_x(x_real, x_imag):
    """[Bc, 1024] f32 -> xt [128, 8192] bf16: p = 8j+n2, free = g*1024+plane*512+s."""
    out = np.empty((NCORES, 128, 8, 2, 512), BF)
    for cid in range(NCORES):
        rows = slice(cid * S, (cid + 1) * S)
        for pi, arr in enumerate((x_real, x_imag)):
            a = arr[rows].reshape(S, 8, 8, 16)          # (s, n2, g, j)
            a = a.transpose(3, 1, 2, 0)                 # (j, n2, g, s)
            out[cid, :, :, pi, :] = a.reshape(128, 8, S).astype(BF)
    return out.reshape(NCORES, 128, 8192)


def _build_wpack(w0r, w0i, wlr, wli):
    wp = np.zeros((128, 10), np.float32)
    wp[:, 0] = w0r[0:128]
    wp[:, 1] = w0i[0:128]
    wp[:, 2] = wlr[0:128]
    wp[:, 3] = wli[0:128]
    wp[:, 4] = wlr[128:256]
    wp[:, 5] = wli[128:256]
    wp[0, 6] = w0r[128]
    wp[0, 7] = w0i[128]
    wp[0, 8] = wlr[256]
    wp[0, 9] = wli[256]
    return wp


def _out_maps():
    """(rows, cols_in_yraw_per_g, out_col) for valid outputs."""
    rr, cc, oo = [], [], []
    for bi, (j0, j1) in enumerate(JBLK):
        for jp in range(j1 - j0):
            for qi in range(17):
                q = qi + 1
                r = jp * 17 + qi
                for g in range(8):
                    n = q * 128 + 16 * g + j0 + jp
                    if CROP0 <= n < CROP0 + CLASS_NUM:
                        rr.append(r)
                        cc.append(g * 1536 + bi * 512)
                        oo.append(n - CROP0)
    return np.array(rr), np.array(cc), np.array(oo)


_OUT_R, _OUT_C, _OUT_O = _out_maps()


def kernel(**inputs):
    global _NC_CACHE
    x_real = np.ascontiguousarray(inputs["x_real"], dtype=np.float32)
    x_imag = np.ascontiguousarray(inputs["x_imag"], dtype=np.float32)
    w0r = np.ascontiguousarray(inputs["w0_real"], dtype=np.float32)
    w0i = np.ascontiguousarray(inputs["w0_imag"], dtype=np.float32)
    wlr = np.ascontiguousarray(inputs["wl_real"], dtype=np.float32)
    wli = np.ascontiguousarray(inputs["wl_imag"], dtype=np.float32)

    xts = _host_x(x_real, x_imag)
    wp = _build_wpack(w0r, w0i, wlr, wli)

    const_maps = {nm: np.ascontiguousarray(arr) for nm, arr in CONSTS.items()}
    in_maps = []
    for cid in range(NCORES):
        m = {"xt": np.ascontiguousarray(xts[cid]), "wpack": wp}
        m.update(const_maps)
        in_maps.append(m)

    if _NC_CACHE is None:
        _NC_CACHE = build_nc()
    res = run_bass_kernel_spmd(_NC_CACHE, in_maps, core_ids=list(range(NCORES)))

    out = np.empty((B, CLASS_NUM), np.float32)
    for cid in range(NCORES):
        yr = np.asarray(res.results[cid]["yraw"], dtype=np.float32)  # [119, 12288]
        # gather: out[s, oo] = yr[rr, cc + s]
        sub = yr[_OUT_R[:, None], _OUT_C[:, None] + np.arange(S)[None, :]]  # [nv, S]
        out[cid * S : (cid + 1) * S, _OUT_O] = sub.T
    return out
